# revision 1
# baseline (speedup 1.0000x reference)
"""Trainium2 Bass kernel for nn_LundWeight (Lund fragmentation reweighting).

Math (per event b, particle m, trial k), matching reference.py:
  fe_s(z; m) = K_s - E_s/z - log z + a_s*log(1-z),   E_s = b_s*mT^2
  K_s = E_s/zmax_s + log zmax_s - a_s*log(1-zmax_s)
  acc (k=0):   d0 = clip(fe_n,-10,10) - clip(fe_b,-10,10)        [log acc_w]
  rej (k>=1):  log rej_w = log(1-G_n) - log(1-G_b),  G_s = exp(fe_s)/15
  weights[b] = exp( sum_m d0 + sum_{m,k>=1} log rej_w )

Device strategy (per core, 1024 events, data-parallel over 8 cores):
  partition dim = event-within-chunk (128), free dim = (m,k) = 2176.
  Omega_s = (K_s - log15 | poisoned to -1e6 where m>=obs) - b_s*(mT^2*r)
            + a_s*l1 - l0     with r = 1/z via exp(-l0), l0 = ln(z+1e-30)
  Masked entries (z==0 or m>=obs) drive Omega so negative that
  exp(Omega) == 0.0 exactly, so they contribute exactly 0 to every sum
  (acc column: both clips saturate to the same bound -> d0 == 0).
  log(1-G) runs on ACT with a fused accum_out reduction.

The two scalar params are baked into the compiled program (recompiled per
distinct value; the program itself handles any general-branch values).
"""

import math
import os
import sys

sys.path.insert(0, "/opt/trn_rl_repo")

import numpy as np

PARAMS_BASE_A = 0.72
PARAMS_BASE_B = 0.88
OVER_SAMPLE = 15.0
AFROMZERO = 0.02
AFROMC = 0.01
EXPMAX = 10.0

N_CORES = 8
B_FULL, M, K = 8192, 128, 17
B_LOCAL = B_FULL // N_CORES          # 1024
NB = B_LOCAL // 128                  # 8 chunks of 128 events
MK = M * K                           # 2176

L15 = math.log(OVER_SAMPLE)
BIG = 1.0e6
DELTA = 1e-30                        # ln bias: z==0 -> l0=-69, r=1e30 -> exp underflow

_CACHE: dict = {}


def _emit(nc, tc, tile, mybir, aps, a_n, b_n, a_b, b_b, fast, reps=1):
    """Emit the per-core program. aps: dict of DRAM APs."""
    Alu = mybir.AluOpType
    Act = mybir.ActivationFunctionType
    f32 = mybir.dt.float32
    i32 = mybir.dt.int32

    zf, mt, obs, wout = aps["zf"], aps["mt"], aps["obs"], aps["wout"]

    sets = (("n", a_n, b_n), ("b", a_b, b_b))

    with tc.tile_pool(name="persist", bufs=1) as pp:
        # ---------------- phase 0: per-(event, m) precompute -------------
        # Wait-slot discipline: TPB compute instructions (ACT/DVE) can carry
        # only ONE semaphore wait, so every ACT/DVE op must need at most one
        # unobserved cross-engine dependency. Strided DMAs and mask compares
        # run on gpsimd (software queue, no such limit).
        mt_all = pp.tile([128, NB, M], f32, tag="mt_all")
        nc.sync.dma_start(out=mt_all, in_=mt.rearrange("(c p) m -> p c m", p=128))
        obs_all = pp.tile([128, NB], f32, tag="obs_all")
        nc.gpsimd.dma_start(out=obs_all, in_=obs.rearrange("(c p) o -> p (c o)", p=128))

        mt2 = pp.tile([128, NB, M], f32, tag="mt2")
        nc.scalar.activation(mt2, mt_all, Act.Square)
        mt2f = mt2.rearrange("p c m -> p (c m)")

        cdelta = pp.tile([128, 1], f32, tag="cdelta")
        nc.gpsimd.memset(cdelta, DELTA)

        wp = {}
        with tc.tile_pool(name="ph0", bufs=1) as p0:
            iota_f = p0.tile([128, M], f32, tag="iota_f")
            nc.gpsimd.iota(
                iota_f, pattern=[[1, M]], base=0, channel_multiplier=0,
                allow_small_or_imprecise_dtypes=True,
            )
            mm_all = p0.tile([128, NB, M], f32, tag="mm_all")
            nm_all = p0.tile([128, NB, M], f32, tag="nm_all")
            for c in range(NB):
                ob = obs_all[:, c : c + 1]
                nc.vector.tensor_scalar(mm_all[:, c, :], iota_f, ob, None, Alu.is_lt)
                nc.vector.tensor_scalar(nm_all[:, c, :], iota_f, ob, None, Alu.is_ge)
            mmf = mm_all.rearrange("p c m -> p (c m)")
            nmf = nm_all.rearrange("p c m -> p (c m)")

            for tag, a_s, b_s in sets:
                if fast:
                    c1 = 0.5 / (1.0 - a_s)
                    # disc^2 = (E-1)^2 + 4aE = E^2 + (4a-2)E + 1, E = b*mT2
                    sqE = p0.tile([128, NB * M], f32, tag="sqE", name="sqE")
                    nc.scalar.activation(sqE, mt2f, Act.Square, scale=b_s)
                    v = p0.tile([128, NB * M], f32, tag="v")
                    nc.vector.scalar_tensor_tensor(
                        v, mt2f, (4.0 * a_s - 2.0) * b_s, sqE, Alu.mult, Alu.add
                    )
                    lv = p0.tile([128, NB * M], f32, tag="lv")
                    nc.scalar.activation(lv, v, Act.Ln, bias=1.0)
                    disc = p0.tile([128, NB * M], f32, tag="disc")
                    nc.scalar.activation(disc, lv, Act.Exp, scale=0.5)
                    # zmax = c1*(E + 1 - disc)
                    u = p0.tile([128, NB * M], f32, tag="u")
                    nc.vector.tensor_scalar(
                        u, mt2f, c1 * b_s, c1, Alu.mult, Alu.add
                    )
                    zg = p0.tile([128, NB * M], f32, tag="zg")
                    nc.vector.scalar_tensor_tensor(
                        zg, disc, -c1, u, Alu.mult, Alu.add
                    )
                    # lzp = log(zmax) - log 15 ; izp = 15/zmax
                    lzp = p0.tile([128, NB * M], f32, tag="lzp")
                    nc.scalar.activation(lzp, zg, Act.Ln, scale=1.0 / OVER_SAMPLE)
                    izp = p0.tile([128, NB * M], f32, tag="izp")
                    nc.scalar.activation(izp, lzp, Act.Exp, scale=-1.0)
                    # k2 = E/zmax + log(zmax) - a*log(1-zmax) - log15
                    w_t = p0.tile([128, NB * M], f32, tag="w_t")
                    nc.vector.tensor_mul(w_t, mt2f, izp)
                    k1 = p0.tile([128, NB * M], f32, tag="k1")
                    nc.vector.scalar_tensor_tensor(
                        k1, w_t, b_s / OVER_SAMPLE, lzp, Alu.mult, Alu.add
                    )
                    l1m = p0.tile([128, NB * M], f32, tag="l1m")
                    nc.scalar.activation(l1m, zg, Act.Ln, bias=1.0, scale=-1.0)
                    k2 = p0.tile([128, NB * M], f32, tag="k2")
                    nc.vector.scalar_tensor_tensor(
                        k2, l1m, -a_s, k1, Alu.mult, Alu.add
                    )
                else:
                    k2 = p0.tile([128, NB, M], f32, tag="k2")
                    nc.sync.dma_start(
                        out=k2,
                        in_=aps["wp" + tag].rearrange("(c p) m -> p c m", p=128),
                    )
                    k2 = k2.rearrange("p c m -> p (c m)")

                # poison: wp = k2*mm - BIG*(1-mm)  (exact where valid);
                # x1 on gpsimd so it can absorb multi-source waits
                x1 = p0.tile([128, NB * M], f32, tag="x1")
                nc.gpsimd.tensor_mul(x1, k2, mmf)
                wp_t = pp.tile([128, NB, M], f32, tag=f"wp_{tag}", name=f"wp_{tag}")
                nc.vector.scalar_tensor_tensor(
                    wp_t.rearrange("p c m -> p (c m)"), nmf, -BIG, x1,
                    Alu.mult, Alu.add,
                )
                wp[tag] = wp_t

        # ---------------- phase 1: per-chunk element pipeline ------------
        import contextlib
        with tc.tile_pool(name="pz", bufs=2) as pz, \
             tc.tile_pool(name="pw", bufs=1) as pw, \
             tc.tile_pool(name="ps", bufs=2) as ps, \
             tc.tile_pool(name="ph", bufs=1, space="PSUM") as ph, \
             (tc.For_i(0, reps, 1) if reps > 1 else contextlib.nullcontext()):
            lw_all = pw.tile([128, NB], f32, tag="lw_all")
            for c in range(NB):
                zt = pz.tile([128, MK], f32, tag="zt")
                nc.sync.dma_start(out=zt, in_=zf[c * 128 : (c + 1) * 128, :])

                l0 = pw.tile([128, MK], f32, tag="l0", bufs=2)
                nc.scalar.activation(l0, zt, Act.Ln, bias=cdelta)
                r = pw.tile([128, MK], f32, tag="r", bufs=2)
                nc.scalar.activation(r, l0, Act.Exp, scale=-1.0)
                l1 = pw.tile([128, MK], f32, tag="l1", bufs=2)
                nc.scalar.activation(l1, zt, Act.Ln, bias=1.0, scale=-1.0)

                # P = mT2 * r  (broadcast mT2 over k)
                P = pw.tile([128, MK], f32, tag="P")
                mt2c = mt2[:, c, :].unsqueeze(2).broadcast_to([128, M, K])
                nc.vector.tensor_mul(
                    P.rearrange("p (m k) -> p m k", k=K),
                    r.rearrange("p (m k) -> p m k", k=K),
                    mt2c,
                )

                om = {}
                t2 = {}
                for tag, a_s, b_s in sets:
                    t2[tag] = pw.tile([128, MK], f32, tag=f"t2{tag}", name=f"t2{tag}")
                    nc.vector.scalar_tensor_tensor(
                        t2[tag], l1, a_s, l0, Alu.mult, Alu.subtract
                    )
                base_pool = os.environ.get("LUND_BASE_ENG", "dve") == "pool"
                for tag, a_s, b_s in sets:
                    add_eng = nc.gpsimd if (base_pool and tag == "b") else nc.vector
                    t3 = pw.tile([128, MK], f32, tag=f"t3{tag}")
                    nc.vector.scalar_tensor_tensor(
                        t3, P, -b_s, t2[tag], Alu.mult, Alu.add
                    )
                    o = pw.tile([128, MK], f32, tag=f"om{tag}", bufs=2)
                    wpc = wp[tag][:, c, :].unsqueeze(2).broadcast_to([128, M, K])
                    add_eng.tensor_add(
                        o.rearrange("p (m k) -> p m k", k=K),
                        t3.rearrange("p (m k) -> p m k", k=K),
                        wpc,
                    )
                    om[tag] = o

                g = {}
                for tag, a_s, b_s in sets:
                    gt = pw.tile([128, MK], f32, tag=f"g{tag}")
                    nc.scalar.activation(gt, om[tag], Act.Exp)
                    g[tag] = gt

                # rej: sum log(1-G) over (m, k>=1)
                s_rej = {}
                for tag, a_s, b_s in sets:
                    hs = ph.tile([128, M, K - 1], f32, tag=f"h{tag}", name=f"h{tag}")
                    acc = ps.tile([128, 1], f32, tag=f"s{tag}")
                    g3 = g[tag].rearrange("p (m k) -> p m k", k=K)
                    nc.scalar.activation(
                        hs, g3[:, :, 1:K], Act.Ln, bias=1.0, scale=-1.0,
                        accum_out=acc,
                    )
                    s_rej[tag] = acc

                # acc column: d0 = clip(om_n) - clip(om_b), summed over m
                cl = {}
                for tag, a_s, b_s in sets:
                    ct = pw.tile([128, M], f32, tag=f"c{tag}")
                    o3 = om[tag].rearrange("p (m k) -> p m k", k=K)
                    nc.vector.tensor_scalar(
                        ct, o3[:, :, 0], -EXPMAX - L15, EXPMAX - L15,
                        Alu.max, Alu.min,
                    )
                    cl[tag] = ct
                d0 = pw.tile([128, M], f32, tag="d0")
                nc.vector.tensor_sub(d0, cl["n"], cl["b"])
                s0 = ps.tile([128, 1], f32, tag="s0")
                nc.vector.tensor_reduce(s0, d0, mybir.AxisListType.X, Alu.add)

                q = ps.tile([128, 1], f32, tag="q")
                nc.vector.tensor_sub(q, s_rej["n"], s_rej["b"])
                nc.vector.tensor_add(lw_all[:, c : c + 1], q, s0)

            wv = pw.tile([128, NB], f32, tag="wv")
            nc.scalar.activation(wv, lw_all, Act.Exp)
            nc.gpsimd.dma_start(
                out=wout.rearrange("(c p) -> p c", p=128), in_=wv
            )


def _build(a_n, b_n, a_b, b_b, fast, reps=1):
    import concourse.bacc as bacc
    import concourse.mybir as mybir
    import concourse.tile as tile
    import bass_rust as _bass_rust
    from concourse.hw_specs import get_activation_tables

    class _Bacc(bacc.Bacc):
        def insert_act_table_loads(self):
            """All our activation funcs (Ln/Exp/Square/Copy/Identity) live in
            the combined natural_log_exp_and_others set; the default chooser
            alternates natural_log <-> exp_and_others and emits ~45 table
            loads (~2.7us each). Hide the funcs from every other set so one
            load suffices."""
            has_activation = any(
                isinstance(i, mybir.InstActivation)
                for b in self.main_func.blocks
                for i in b.instructions
            )
            if not has_activation:
                return
            tables = list(get_activation_tables(self.m.arch).items())
            target = next(
                i for i, (n, _) in enumerate(tables)
                if n == "natural_log_exp_and_others"
            )
            forced = [
                (n, (funcs if i == target else set()))
                for i, (n, funcs) in enumerate(tables)
            ]
            _bass_rust.insert_act_table_loads(self, forced)

    f32 = mybir.dt.float32
    nc = _Bacc("TRN2", debug=False)
    aps = {}
    aps["zf"] = nc.dram_tensor("zf", [B_LOCAL, MK], f32, kind="ExternalInput").ap()
    aps["mt"] = nc.dram_tensor("mt", [B_LOCAL, M], f32, kind="ExternalInput").ap()
    aps["obs"] = nc.dram_tensor("obs", [B_LOCAL, 1], f32, kind="ExternalInput").ap()
    if not fast:
        aps["wpn"] = nc.dram_tensor(
            "wpn", [B_LOCAL, M], f32, kind="ExternalInput"
        ).ap()
        aps["wpb"] = nc.dram_tensor(
            "wpb", [B_LOCAL, M], f32, kind="ExternalInput"
        ).ap()
    aps["wout"] = nc.dram_tensor("wout", [B_LOCAL], f32, kind="ExternalOutput").ap()

    with tile.TileContext(nc) as tc:
        _emit(nc, tc, tile, mybir, aps, a_n, b_n, a_b, b_b, fast, reps=reps)
    nc.compile()
    return nc


def _host_k2(a_s, b_s, mt2):
    """Reference-faithful K (minus log15) on host, fp64, general for all
    reference branches. mt2: [B, M] float64. Returns K - log15."""
    E = b_s * mt2
    a_is_zero = a_s < AFROMZERO
    a_is_c = abs(a_s - 1.0) < AFROMC
    denom = 1.0 if (a_is_zero or a_is_c) else (1.0 - a_s)
    disc = np.sqrt((E - 1.0) ** 2 + 4.0 * a_s * E)
    z_gen = 0.5 * (E + 1.0 - disc) / denom
    z_gen = np.where(
        (z_gen > 0.9999) & (E > 100.0), np.minimum(z_gen, 1.0 - a_s / E), z_gen
    )
    if a_is_zero:
        zmax = np.where(1.0 > E, E, 1.0)
    elif a_is_c:
        zmax = E / (E + 1.0)
    else:
        zmax = z_gen
    K2 = E / zmax + np.log(zmax)
    if not a_is_zero:
        K2 = K2 - a_s * np.log1p(-zmax)
    return K2 - L15


def _fast_ok(a_n, b_n, a_b, b_b, mt2max):
    for a_s, b_s in ((a_n, b_n), (a_b, b_b)):
        if a_s < AFROMZERO or abs(a_s - 1.0) < AFROMC:
            return False
        if b_s * mt2max > 100.0:
            return False  # conservatively avoid the z_gen>0.9999 branch
        # zmax bounds for log safety: zmax in (0,1) guaranteed for the
        # general branch when E>0; E==0 impossible unless b_s==0 or mT==0
        if b_s <= 0.0:
            return False
    return True


def kernel(z, mT, observable, params_a, params_b):
    from concourse import bass_utils

    z = np.ascontiguousarray(np.asarray(z, dtype=np.float32))
    mT = np.ascontiguousarray(np.asarray(mT, dtype=np.float32))
    obs_f = np.asarray(observable).astype(np.float32).reshape(-1, 1)
    a_n = float(np.asarray(params_a))
    b_n = float(np.asarray(params_b))
    a_b, b_b = PARAMS_BASE_A, PARAMS_BASE_B

    B, M_, K_ = z.shape
    assert (B, M_, K_) == (B_FULL, M, K), (B, M_, K_)

    mt2max = float(np.max(mT.astype(np.float64)) ** 2)
    fast = _fast_ok(a_n, b_n, a_b, b_b, mt2max)
    # masked-entry exactness: need b_s*min(mT^2)/DELTA to drive exp to 0
    mt2min = float(np.min(mT.astype(np.float64)) ** 2)
    if fast and min(b_n, b_b) * mt2min / DELTA < 1e4:
        fast = False

    key = (a_n, b_n, a_b, b_b, fast)
    if key not in _CACHE:
        _CACHE[key] = _build(a_n, b_n, a_b, b_b, fast)
    nc = _CACHE[key]

    zf = z.reshape(B, MK)
    in_maps = []
    for cidx in range(N_CORES):
        lo, hi = cidx * B_LOCAL, (cidx + 1) * B_LOCAL
        m = {
            "zf": zf[lo:hi],
            "mt": mT[lo:hi],
            "obs": obs_f[lo:hi],
        }
        if not fast:
            mt2 = mT[lo:hi].astype(np.float64) ** 2
            mask = np.arange(M)[None, :] < np.asarray(observable[lo:hi]).reshape(
                -1, 1
            )
            for tag, a_s, b_s in (("n", a_n, b_n), ("b", a_b, b_b)):
                k2 = _host_k2(a_s, b_s, mt2)
                m["wp" + tag] = np.where(mask, k2, -BIG).astype(np.float32)
        in_maps.append(m)

    res = bass_utils.run_bass_kernel_spmd(nc, in_maps, core_ids=list(range(N_CORES)))
    out = np.concatenate([res.results[c]["wout"] for c in range(N_CORES)])
    return out.astype(np.float32)


if __name__ == "__main__":
    # smoke test with random data
    rng = np.random.default_rng(0)
    z = rng.uniform(1e-3, 0.999, size=(B_FULL, M, K)).astype(np.float32)
    z *= rng.random(z.shape) < 0.5
    mT = rng.uniform(0.5, 2.5, size=(B_FULL, M)).astype(np.float32)
    obs = rng.integers(0, M, size=(B_FULL,)).astype(np.int32)
    w = kernel(z, mT, obs, np.float32(0.68), np.float32(0.98))
    print(w[:8])

